# revision 2
# baseline (speedup 1.0000x reference)
"""Trainium2 Bass kernel v2: 3x3x64->1 valid conv over (512, 512, 64), two-pass.

out[r, c] = sum_{fi,fj,d} x[r+fi, c+fj, d] * W[0, (fi*3+fj)*64+d] + b[0]
Output: (510*510,) float32.

Strategy (8-way row sharding, 64 output rows per core + 2-row halo):
  Pass 1 (PE, bf16): per row-pair tile [128=(2 rows x 64 d), 512 cols], one
    matmul contracting depth into 18 channels u_{parity,fi,fj}[c]. 7 pairs
    share one PSUM bank [126, 512] via zero-padded block stationaries Z_jl
    (PSUM matmul outputs are anchored at partition 0, so each pair-slot's
    stationary is a [128, 126] matrix with S at column block jl).
  Copy (ScalarE): per tile, PSUM -> SBUF bf16.
  Pass 2 (PE, bf16): per tile and column shift fj, one matmul with a 0/1
    band stationary gathers u[r+fi, c+fj, (fi,fj)] into out PSUM [64, 510],
    accumulating over a DVE-zeroed bank.
  Bias+copy (ScalarE) and output DMA are chunked 5 ways by finalized rows so
    only the last tile's chain sits on the tail.
  Inputs are cast to bf16 on the host (halves HBM traffic; rel err ~5e-3
    vs the 2e-2 gate). All sync is hand-rolled: one wait per instruction.
"""

from contextlib import ExitStack

import numpy as np
import ml_dtypes

import concourse.bass as bass
import concourse.mybir as mybir
from concourse.bass_utils import run_bass_kernel_spmd

N_CORES = 8
H = 512
WD = 512
D = 64
NOUT = 510
R_PER_CORE = 64           # output rows per core (last 2 of core 7 discarded)
ROWS_IN = R_PER_CORE + 2  # input rows per core incl. halo
NPAIRS = ROWS_IN // 2     # 33

TSIZES = [7, 7, 7, 7, 5]              # row-pairs per PSUM tile
TSTARTS = [0, 7, 14, 21, 28]
GSIZES = [1, 2, 4, 7, 7, 7, 4, 1]     # row-pairs per input DMA
GSTARTS = [sum(GSIZES[:i]) for i in range(len(GSIZES))]
# pass-2 band widths per (t, fj): the very first pass-2 matmul opens the
# PSUM accumulation group with start=True so it must span all 64 rows
# (its zero columns write the zeros); tile 4 is naturally 64 wide.
BW = [[64 if (t == 0 and fj == 0) or t == 4 else 14 * t + 14
       for fj in range(3)] for t in range(5)]
BWFLAT = [w for row in BW for w in row]
# single output chunk: acc is readable only after its accumulation group
# closes (stop=True on the last pass-2 matmul)
CHUNKS = [(0, 64)]

SOFF = WD                       # S block at cols [512, 530)
BOFF = WD + 18                  # B blocks after S
G0COLS = BOFF + sum(BWFLAT)

DT = mybir.dt.bfloat16

assert sum(TSIZES) == NPAIRS and sum(GSIZES) == NPAIRS


def _pair_group(j):
    for g, gsz in enumerate(GSIZES):
        if GSTARTS[g] <= j < GSTARTS[g] + gsz:
            return g, j - GSTARTS[g]
    raise AssertionError


def _boff(t, fj):
    return BOFF + sum(BWFLAT[:3 * t + fj])


def _build_nc(bias_val: float) -> bass.Bass:
    nc = bass.Bass()
    xg_dram = []
    for gi, gsz in enumerate(GSIZES):
        cols = G0COLS if gi == 0 else gsz * WD
        xg_dram.append(nc.dram_tensor(f"x{gi}", [128, cols], DT,
                                      kind="ExternalInput"))
    out = nc.dram_tensor("out", [R_PER_CORE, NOUT], mybir.dt.float32,
                         kind="ExternalOutput")

    with ExitStack() as ctx:
        tiles = []
        for gi, gsz in enumerate(GSIZES):
            cols = G0COLS if gi == 0 else gsz * WD
            tiles.append(ctx.enter_context(
                nc.sbuf_tensor(f"xg{gi}", [128, cols], DT)))
        zsb = ctx.enter_context(nc.sbuf_tensor("zsb", [128, 7 * 126], DT))
        usb = [ctx.enter_context(nc.sbuf_tensor(f"usb{t}", [126, WD], DT))
               for t in range(5)]
        osb = ctx.enter_context(
            nc.sbuf_tensor("osb", [R_PER_CORE, NOUT], mybir.dt.float32))
        upsum = [ctx.enter_context(nc.psum_tensor(f"u{t}", [126, WD],
                                                  mybir.dt.float32))
                 for t in range(5)]
        acc = ctx.enter_context(
            nc.psum_tensor("acc", [R_PER_CORE, NOUT], mybir.dt.float32))

        dma_sems = [ctx.enter_context(nc.semaphore(f"dma_sem{gi}"))
                    for gi in range(len(GSIZES))]
        zm_sem = ctx.enter_context(nc.semaphore("zm_sem"))
        zc_sem = ctx.enter_context(nc.semaphore("zc_sem"))
        p1_sem = ctx.enter_context(nc.semaphore("p1_sem"))
        cp_sem = ctx.enter_context(nc.semaphore("cp_sem"))
        p2_sem = ctx.enter_context(nc.semaphore("p2_sem"))
        a_sem = ctx.enter_context(nc.semaphore("a_sem"))
        out_sem = ctx.enter_context(nc.semaphore("out_sem"))
        block = ctx.enter_context(nc.Block())

        t0 = tiles[0]

        @block.sync
        def _(sync):
            for gi in range(len(GSIZES)):
                sync.dma_start(tiles[gi][:, :], xg_dram[gi][:, :]) \
                    .then_inc(dma_sems[gi], 16)
            sync.wait_ge(a_sem, 1)
            sync.dma_start(out[:, :], osb[:, :]).then_inc(out_sem, 16)

        @block.vector
        def _(vector):
            # Build Z on-device: Z_jl = S at column block jl of a zeroed
            # [128, 126] matrix, at zsb[:, jl*126 : (jl+1)*126].
            nc.vector.memset(zsb[:, :], 0.0).then_inc(zm_sem, 1)
            vector.wait_ge(zm_sem, 1)
            vector.wait_ge(dma_sems[0], 16)
            cpi = None
            for jl in range(7):
                cpi = nc.vector.tensor_copy(
                    zsb[:, jl * 126 + jl * 18: jl * 126 + (jl + 1) * 18],
                    t0[:, SOFF: SOFF + 18])
            cpi.then_inc(zc_sem, 1)

        @block.tensor
        def _(tensor):
            cur_g = -1
            tensor.wait_ge(zc_sem, 1)

            def p1_tile(t):
                nonlocal cur_g
                ts = TSIZES[t]
                for jl in range(ts):
                    j = TSTARTS[t] + jl
                    g, l = _pair_group(j)
                    if g != cur_g:
                        tensor.wait_ge(dma_sems[g], 16)
                        cur_g = g
                    mm = nc.tensor.matmul(
                        upsum[t][:, :],
                        lhsT=zsb[:, jl * 126: (jl + 1) * 126],
                        rhs=tiles[g][:, l * WD: (l + 1) * WD],
                        start=(jl == 0),
                        stop=(jl == ts - 1),
                    )
                    if jl == ts - 1:
                        mm.then_inc(p1_sem, 1)

            def p2_tile(t):
                ts = TSIZES[t]
                tensor.wait_ge(cp_sem, t + 1)
                for fj in range(3):
                    w = BW[t][fj]
                    bo = _boff(t, fj)
                    mm = nc.tensor.matmul(
                        acc[0:w, :],
                        lhsT=t0[0:ts * 18, bo: bo + w],
                        rhs=usb[t][0:ts * 18, fj: fj + NOUT],
                        start=(t == 0 and fj == 0),
                        stop=(t == 4 and fj == 2),
                    )
                    if fj == 2:
                        mm.then_inc(p2_sem, 1)

            p1_tile(0)
            p1_tile(1)
            p2_tile(0)
            p1_tile(2)
            p2_tile(1)
            p1_tile(3)
            p2_tile(2)
            p1_tile(4)
            p2_tile(3)
            p2_tile(4)

        @block.scalar
        def _(scalar):
            def cp(t):
                scalar.wait_ge(p1_sem, t + 1)
                nc.scalar.activation(usb[t][0:TSIZES[t] * 18, :],
                                     upsum[t][0:TSIZES[t] * 18, :],
                                     mybir.ActivationFunctionType.Copy,
                                     bias=0.0, scale=1.0) \
                    .then_inc(cp_sem, 1)

            def bias_chunk(t):
                rlo, rhi = CHUNKS[t]
                scalar.wait_ge(p2_sem, 5)
                nc.scalar.activation(osb[rlo:rhi, :], acc[rlo:rhi, :],
                                     mybir.ActivationFunctionType.Copy,
                                     bias=float(bias_val), scale=1.0) \
                    .then_inc(a_sem, 1)

            cp(0)
            cp(1)
            cp(2)
            cp(3)
            cp(4)
            bias_chunk(0)

    return nc


def _prep_inputs(x: np.ndarray, W: np.ndarray):
    xt = np.ascontiguousarray(x.transpose(0, 2, 1))  # (512, 64, 512)
    xt_pad = np.zeros((N_CORES * R_PER_CORE + 2, D, WD), np.float32)
    xt_pad[:H] = xt

    w = np.asarray(W, np.float32)[0].reshape(3, 3, D)

    # S[parity*64+d, parity*9+k] = w[fi, fj, d], k = 3*fi+fj
    S = np.zeros((128, 18), np.float32)
    for parity in range(2):
        for fi in range(3):
            for fj in range(3):
                k = 3 * fi + fj
                S[parity * 64:(parity + 1) * 64, parity * 9 + k] = w[fi, fj]

    # B_{t,fj}: [128, w_t] 0/1 band gathering u[r+fi, ., (fi,fj)] into row r
    Bs = []
    for t in range(5):
        for fj in range(3):
            wt = BW[t][fj]
            mat = np.zeros((128, wt), np.float32)
            for jl in range(TSIZES[t]):
                for parity in range(2):
                    for fi in range(3):
                        k = 3 * fi + fj
                        p = jl * 18 + parity * 9 + k
                        r = 2 * (TSTARTS[t] + jl) + parity - fi
                        if 0 <= r < wt:
                            mat[p, r] = 1.0
            Bs.append(mat)
    B = np.concatenate(Bs, axis=1)

    extras = np.concatenate([S, B], axis=1)

    in_maps = []
    for i in range(N_CORES):
        shard = xt_pad[R_PER_CORE * i: R_PER_CORE * i + ROWS_IN]
        pairs = shard.reshape(NPAIRS, 2, D, WD)
        m = {}
        for gi, gsz in enumerate(GSIZES):
            j0 = GSTARTS[gi]
            # [gsz, 2, 64, 512] -> [(2, 64)=partition, gsz*512]
            blk = pairs[j0:j0 + gsz].transpose(1, 2, 0, 3).reshape(128, gsz * WD)
            if gi == 0:
                blk = np.concatenate([blk, extras], axis=1)
            m[f"x{gi}"] = np.ascontiguousarray(blk).astype(ml_dtypes.bfloat16)
        in_maps.append(m)
    return in_maps


def kernel(x: np.ndarray, W: np.ndarray, b: np.ndarray, _trace=False):
    x = np.asarray(x, np.float32)
    in_maps = _prep_inputs(x, W)
    nc = _build_nc(float(np.asarray(b).reshape(-1)[0]))
    res = run_bass_kernel_spmd(nc, in_maps, core_ids=list(range(N_CORES)),
                               trace=_trace)
    full = np.concatenate([res.results[i]["out"] for i in range(N_CORES)], 0)
    out = full[:NOUT].reshape(-1).astype(np.float32)
    if _trace:
        return out, res
    return out
